# revision 16
# baseline (speedup 1.0000x reference)
"""Trainium2 Bass kernel for nn_ActorNetwork (GNN message passing).

Strategy (8 NeuronCores, SPMD):
  - Row-range sharding: core k owns node rows [k*25000, (k+1)*25000) and all
    edges targeting them.
  - GCN norm folds into node-wise scales: aggr = Dinv (A+I) Dinv h, so
    g = dinv*h is pre-scaled before the gather and dinv is applied after the
    segment-sum. No per-edge norm work.
  - Host prep (structural only): cast indices to int32, CSR by row, per-core
    degree sort (uniform per-block padded degree), common padded block table
    across cores (SPMD), self-loops inserted, pad slots point at a zero row.
  - Device: mlp1 -> g table shard -> AllGather -> indirect-DMA gather of
    padded slots -> strided DVE segment reduce -> dinv scale -> mlp2 ->
    un-permute via DRAM round trip -> per-dag sums -> mlp_dag -> AllReduce
    (32 floats) -> mlp_glob -> ops scores + prlvl scores.
"""
import math
import sys

sys.path.insert(0, "/opt/trn_rl_repo")

import numpy as np

import concourse.bass as bass
import concourse.bacc as bacc
import concourse.mybir as mybir
import concourse.tile as tile
from concourse.bass_utils import run_bass_kernel_spmd
from concourse.masks import make_identity
from concourse.tile_rust import add_dep_helper

F32 = mybir.dt.float32
I32 = mybir.dt.int32
AX = mybir.AxisListType
ALU = mybir.AluOpType
ACT_T = mybir.ActivationFunctionType


class Cfg:
    def __init__(self, N=200000, E=6400000, IN=16, D=32, G=1000, W=50, NC=8):
        self.N, self.E, self.IN, self.D, self.G, self.W, self.NC = N, E, IN, D, G, W, NC
        self.RC = N // NC                      # rows per core
        self.SBLK = -(-self.RC // 512)         # 128-row blocks per packing slice
        self.NBLK = 4 * self.SBLK              # total 128-row blocks (padded)
        self.RCP = self.NBLK * 128             # padded rows per core
        self.PCK = self.RCP // 4               # columns per packing slice
        self.DPC = G // NC                     # dags per core
        self.OPP = N // G                      # nodes per dag
        self.SDAG = -(-self.DPC // 4)          # dags per output slice
        self.OPSW = self.SDAG * self.OPP       # cols per output slice
        self.PADROW = NC * self.RCP            # zero row index in gathered table
        self.TROWS = self.PADROW + 8           # table rows
        self.CW = 512                          # matmul chunk width
        self.CWG = 128                         # gather chunk width (idx cols)


def _blk4(w):
    return np.kron(np.eye(4, dtype=np.float32), np.asarray(w, np.float32))


def _t4(b):
    return np.tile(np.asarray(b, np.float32), 4)[:, None].copy()


def _prep(cfg, x, edge_index, weights):
    """Host-side structural prep. Returns per-core input maps + common tables."""
    c = cfg
    x = np.asarray(x, np.float32)
    ei = np.asarray(edge_index)
    row = ei[0].astype(np.int64)
    col = ei[1].astype(np.int64)

    deg = (np.bincount(col, minlength=c.N) + 1.0).astype(np.float32)  # out-deg + self
    indeg = np.bincount(row, minlength=c.N).astype(np.int64) + 1      # slots per row

    order = np.argsort(row, kind="stable")
    col_s = col[order]
    rowptr = np.zeros(c.N + 1, np.int64)
    rowptr[1:] = np.cumsum(np.bincount(row, minlength=c.N))
    colmap = ((col_s // c.RC) * c.RCP + (col_s % c.RC)).astype(np.int64)

    perms = []
    kb_per_core = np.zeros((c.NC, c.NBLK), np.int64)
    for k in range(c.NC):
        base = k * c.RC
        ik = indeg[base:base + c.RC]
        perm = np.argsort(-ik, kind="stable")
        perms.append(perm)
        ik_pad = np.zeros(c.RCP, np.int64)
        ik_pad[:c.RC] = ik[perm]
        kb_per_core[k] = ik_pad.reshape(c.NBLK, 128).max(axis=1)
    kb = kb_per_core.max(axis=0)
    kb = np.maximum(kb, 1)
    totk = int(kb.sum())
    col_off = np.zeros(c.NBLK + 1, np.int64)
    col_off[1:] = np.cumsum(kb)

    # gather chunk table: greedy pack blocks with sum(kb) <= CWG
    chunks = []  # (b0, nb, coloff, ncols)
    b0 = 0
    while b0 < c.NBLK:
        nb = 0
        ncols = 0
        while b0 + nb < c.NBLK and (nb == 0 or ncols + kb[b0 + nb] <= c.CWG):
            ncols += kb[b0 + nb]
            nb += 1
        chunks.append((b0, nb, int(col_off[b0]), int(ncols)))
        b0 += nb

    per_core = []
    for k in range(c.NC):
        base = k * c.RC
        perm = perms[k]
        inv = np.empty(c.RC, np.int64)
        inv[perm] = np.arange(c.RC)

        idx = np.full((128, totk), c.PADROW, np.int64)
        perm_pad = np.full(c.RCP, -1, np.int64)
        perm_pad[:c.RC] = perm
        for b in range(c.NBLK):
            lanes = perm_pad[b * 128:(b + 1) * 128]
            valid = lanes >= 0
            if not valid.any():
                continue
            lv = lanes[valid]
            gr = base + lv
            off = int(col_off[b])
            kbb = int(kb[b])
            idx[valid, off] = k * c.RCP + lv  # self loop
            lens = (rowptr[gr + 1] - rowptr[gr])
            if kbb > 1:
                ar = np.arange(kbb - 1)
                m2 = ar[None, :] < lens[:, None]
                src = rowptr[gr][:, None] + ar[None, :]
                tmp = np.full((valid.sum(), kbb - 1), c.PADROW, np.int64)
                tmp[m2] = colmap[src[m2]]
                idx[valid, off + 1:off + kbb] = tmp

        # per-node packed degree tables
        nid = np.arange(c.RCP)
        node_pck = (nid % c.PCK) // 128 * 0  # placeholder
        # deg_gb8: [j, ((cb*4)+cslice)*8 + d] = deg[base + cslice*PCK + cb*128 + j]
        j = np.arange(128)
        cb = np.arange(c.SBLK)
        cs = np.arange(4)
        node = (cs[None, None, :] * c.PCK + cb[None, :, None] * 128 + j[:, None, None])
        dg = np.where(node < c.RC, deg[base + np.minimum(node, c.RC - 1)], 1.0)
        deg_gb8 = np.repeat(dg.reshape(128, c.SBLK * 4), 8, axis=1).astype(np.float32)

        # deg_pb: [j, b] = deg of permuted row b*128+j
        pp = np.arange(c.RCP).reshape(c.NBLK, 128).T  # [128, NBLK]
        rloc = perm_pad[pp]
        deg_pb = np.where(rloc >= 0, deg[base + np.maximum(rloc, 0)], 1.0).astype(np.float32)
        deg_pb = deg_pb.reshape(128, 4, c.SBLK).transpose(0, 2, 1).reshape(128, c.NBLK).copy()

        # iperm: [j, cc] = permuted position of original row cc*128+j
        oc = np.arange(c.NBLK)
        onode = oc[None, :] * 128 + j[:, None]
        ipm = np.where(onode < c.RC, inv[np.minimum(onode, c.RC - 1)], 0).astype(np.int32)

        # x packed for mlp1: [64, PCK] (16c+f, n) = x[base + c*PCK + n, f]
        xp = np.zeros((64, c.PCK), np.float32)
        for s in range(4):
            n0 = s * c.PCK
            n1 = min(n0 + c.PCK, c.RC)
            if n1 > n0:
                xp[16 * s:16 * s + 16, :n1 - n0] = x[base + n0:base + n1].T
        # x dag-aligned: [64, OPSW]
        xy = np.zeros((64, c.OPSW), np.float32)
        for s in range(4):
            n0 = s * c.OPSW
            n1 = min(n0 + c.OPSW, c.RC)
            if n1 > n0:
                xy[16 * s:16 * s + 16, :n1 - n0] = x[base + n0:base + n1].T

        per_core.append(dict(
            x_pack=xp, x_y=xy,
            idx=idx.astype(np.int32).copy(),
            iperm=ipm.copy(),
            deg_gb8=deg_gb8.copy(),
            deg_pb=deg_pb.copy(),
        ))

    (p1, p2, pd, pg, po, pp_) = weights
    wts = dict(
        w1b=_blk4(p1[0]), b1p=_t4(p1[1]), w2b=_blk4(p1[2]), b2p=_t4(p1[3]),
        w3b=_blk4(p1[4]), b3p=_t4(p1[5]),
        v1b=_blk4(p2[0]), c1p=_t4(p2[1]), v2b=_blk4(p2[2]), c2p=_t4(p2[3]),
        v3b=_blk4(p2[4]), c3p=_t4(p2[5]),
        d1=np.asarray(pd[0], np.float32), db1=np.asarray(pd[1], np.float32)[:, None].copy(),
        d2=np.asarray(pd[2], np.float32), db2=np.asarray(pd[3], np.float32)[:, None].copy(),
        d3=np.asarray(pd[4], np.float32), db3=np.asarray(pd[5], np.float32)[:, None].copy(),
        g1=np.asarray(pg[0], np.float32), gb1=np.asarray(pg[1], np.float32)[:, None].copy(),
        g2=np.asarray(pg[2], np.float32), gb2=np.asarray(pg[3], np.float32)[:, None].copy(),
        g3=np.asarray(pg[4], np.float32), gb3=np.asarray(pg[5], np.float32)[:, None].copy(),
        oab=_blk4(np.asarray(po[0], np.float32)[0:32]),
        ob=np.asarray(po[0], np.float32)[32:64].copy(),
        oc=np.asarray(po[0], np.float32)[64:96].copy(),
        ob1=np.asarray(po[1], np.float32)[:, None].copy(),
        o2b=_blk4(po[2]), ob2=_t4(po[3]),
        o3b=_blk4(po[4]), ob3=_t4(po[5]),
        p1l=np.asarray(pp_[0], np.float32)[0:1].copy(),
        p1y=np.asarray(pp_[0], np.float32)[1:33].copy(),
        p1z=np.asarray(pp_[0], np.float32)[33:65].copy(),
        pb1=np.asarray(pp_[1], np.float32)[:, None].copy(),
        p2=np.asarray(pp_[2], np.float32), pb2=np.asarray(pp_[3], np.float32)[:, None].copy(),
        p3=np.asarray(pp_[4], np.float32), pb3=np.asarray(pp_[5], np.float32)[:, None].copy(),
    )
    return dict(per_core=per_core, wts=wts, kb=kb, col_off=col_off, totk=totk,
                chunks=chunks, perms=perms, deg=deg)


def _emulate(cfg, prep):
    """Numpy mirror of the device dataflow end-to-end (for validation)."""
    c = cfg
    w = prep["wts"]

    def relu(a):
        return np.maximum(a, 0.0)

    table = np.zeros((c.TROWS, 8), np.float32)
    for k in range(c.NC):
        pc = prep["per_core"][k]
        xp = pc["x_pack"]
        t1 = relu(w["w1b"].T @ xp + w["b1p"])
        t2 = relu(w["w2b"].T @ t1 + w["b2p"])
        hp = w["w3b"].T @ t2 + w["b3p"]
        dinv_g = np.sqrt(1.0 / pc["deg_gb8"])
        g_sb = np.zeros((128, 4, c.SBLK, 8), np.float32)
        for cb in range(c.SBLK):
            v = (hp[:, cb * 128:(cb + 1) * 128].T
                 * dinv_g[:, cb * 32:(cb + 1) * 32]).reshape(128, 4, 8)
            g_sb[:, :, cb, :] = v
        table[k * c.RCP:(k + 1) * c.RCP] = np.transpose(
            g_sb, (1, 2, 0, 3)).reshape(c.RCP, 8)

    xnts, yts = [], []
    ysum_g = np.zeros(32, np.float32)
    for k in range(c.NC):
        pc = prep["per_core"][k]
        gath = table[pc["idx"]]
        aggr = np.zeros((128, c.NBLK * 8), np.float32)
        for b in range(c.NBLK):
            o0, o1 = prep["col_off"][b], prep["col_off"][b + 1]
            s, cc = b // c.SBLK, b % c.SBLK
            col = (cc * 4 + s) * 8
            aggr[:, col:col + 8] = gath[:, o0:o1, :].sum(axis=1)
        dinv_pb = np.sqrt(1.0 / pc["deg_pb"])
        aggr = (aggr.reshape(128, c.NBLK, 8) * dinv_pb[:, :, None]).reshape(128, -1)
        m2 = np.zeros((32, c.PCK), np.float32)
        for cc in range(c.SBLK):
            m2[:, cc * 128:(cc + 1) * 128] = aggr[:, cc * 32:(cc + 1) * 32].T
        u1 = relu(w["v1b"].T @ m2 + w["c1p"])
        u2 = relu(w["v2b"].T @ u1 + w["c2p"])
        xnp_ = w["v3b"].T @ u2 + w["c3p"]
        xn_dram = np.zeros((c.RCP, 32), np.float32)
        for s in range(4):
            for cc in range(c.SBLK):
                xn_dram[s * c.PCK + cc * 128:s * c.PCK + (cc + 1) * 128] = \
                    xnp_[32 * s:32 * s + 32, cc * 128:(cc + 1) * 128].T
        xo_rows = xn_dram[pc["iperm"].T.reshape(-1)]  # [(cc j)] -> [cc*128+j]
        # iperm is [j, cc]; device gathers per q with idx ap [128, SBLK] p-major:
        # flat order = partition-major: for p: for cc: idx[p, cc] -> dest [p, cc, 32]
        xn_t = np.zeros((128, c.OPSW), np.float32)
        for b in range(c.NBLK):
            j = np.arange(128)
            rows = xn_dram[pc["iperm"][:, b]]  # [128, 32] orig nodes b*128+j
            blkv = rows.T  # [32, 128]
            left = 0
            while left < 128:
                n = b * 128 + left
                s = n // c.OPSW
                if s >= 4:
                    break
                ccol = n - s * c.OPSW
                span = min(128 - left, c.OPSW - ccol)
                xn_t[32 * s:32 * s + 32, ccol:ccol + span] = blkv[:, left:left + span]
                left += span
        nreal = c.RC - 3 * c.OPSW
        xn_t[96:128, nreal:] = 0.0
        xnts.append(xn_t)
        r2 = xn_t.reshape(128, c.SDAG, c.OPP).sum(axis=2)
        r1 = pc["x_y"].reshape(64, c.SDAG, c.OPP).sum(axis=2)
        yin = np.zeros((48, c.DPC), np.float32)
        for s in range(4):
            nd = min(c.SDAG, c.DPC - s * c.SDAG)
            if nd <= 0:
                continue
            yin[0:16, s * c.SDAG:s * c.SDAG + nd] = r1[16 * s:16 * s + 16, :nd]
            yin[16:48, s * c.SDAG:s * c.SDAG + nd] = r2[32 * s:32 * s + 32, :nd]
        q1 = relu(w["d1"].T @ yin + w["db1"])
        q2 = relu(w["d2"].T @ q1 + w["db2"])
        y_t = w["d3"].T @ q2 + w["db3"]
        yts.append(y_t)
        ysum_g += y_t.sum(axis=1)

    e1 = relu(w["g1"].T @ ysum_g[:, None] + w["gb1"])
    e2 = relu(w["g2"].T @ e1 + w["gb2"])
    z_t = w["g3"].T @ e2 + w["gb3"]

    ops_all, prl_all = [], []
    for k in range(c.NC):
        xn_t, y_t = xnts[k], yts[k]
        ybp = w["ob"].T @ y_t                     # [32, DPC]
        opb = (w["oc"].T @ z_t + w["ob1"])        # [32, 1]
        opbr = np.tile(opb, (4, 1))
        ops_sb = np.zeros((4, c.OPSW), np.float32)
        for i in range(c.OPSW // (2 * c.OPP)):
            c0 = i * 2 * c.OPP
            cw = 2 * c.OPP
            ps = w["oab"].T @ xn_t[:, c0:c0 + cw]
            for s in range(4):
                d0 = s * c.SDAG + i * 2
                nds = max(0, min(2, c.DPC - d0))
                if nds > 0:
                    ps[32 * s:32 * s + 32, :nds * c.OPP] += np.repeat(
                        ybp[:, d0:d0 + nds], c.OPP, axis=1)
            t1 = relu(ps + opbr)
            t2 = relu(w["o2b"].T @ t1 + w["ob2"])
            ops_sb[:, c0:c0 + cw] = w["o3b"].T @ t2 + w["ob3"]
        ops_all.append(ops_sb.reshape(-1)[:c.RC])

        pyp = w["p1y"].T @ y_t                    # [32, DPC]
        pb1f = w["p1z"].T @ z_t + w["pb1"]
        prl = np.zeros((1, c.DPC * c.W), np.float32)
        lims = np.arange(1, c.W + 1, dtype=np.float32)
        for i in range(-(-c.DPC // 10)):
            d0 = i * 10
            nd = min(10, c.DPC - d0)
            c0 = d0 * c.W
            cw = nd * c.W
            lim = np.tile(lims, nd)[None, :]
            ps = w["p1l"].T @ lim
            ps += np.repeat(pyp[:, d0:d0 + nd], c.W, axis=1)
            t1 = relu(ps + pb1f)
            t2 = relu(w["p2"].T @ t1 + w["pb2"])
            prl[:, c0:c0 + cw] = w["p3"].T @ t2 + w["pb3"]
        prl_all.append(prl.reshape(c.DPC, c.W))

    return np.concatenate(ops_all), np.concatenate(prl_all)


def _ceil_chunks(total, cw):
    out = []
    o = 0
    while o < total:
        out.append((o, min(cw, total - o)))
        o += cw
    return out


def _build(cfg, prep):
    c = cfg
    w = prep["wts"]
    kb, col_off, chunks, totk = prep["kb"], prep["col_off"], prep["chunks"], prep["totk"]

    nc = bacc.Bacc("TRN2", num_devices=c.NC)

    # ---- parameters
    xp_d = nc.declare_dram_parameter("x_pack", [64, c.PCK], F32, isOutput=False)
    xy_d = nc.declare_dram_parameter("x_y", [64, c.OPSW], F32, isOutput=False)
    idx_d = nc.declare_dram_parameter("idx", [128, totk], I32, isOutput=False)
    ipm_d = nc.declare_dram_parameter("iperm", [128, c.NBLK], I32, isOutput=False)
    dgg_d = nc.declare_dram_parameter("deg_gb8", [128, c.SBLK * 32], F32, isOutput=False)
    dgp_d = nc.declare_dram_parameter("deg_pb", [128, c.NBLK], F32, isOutput=False)
    wd = {}
    for name, arr in w.items():
        wd[name] = nc.declare_dram_parameter("w_" + name, list(arr.shape), F32, isOutput=False)
    zrow_d = nc.declare_dram_parameter("zrow", [1, 8], F32, isOutput=False)
    ops_d = nc.declare_dram_parameter("ops_out", [4, c.OPSW], F32, isOutput=True)
    prl_d = nc.declare_dram_parameter("prl_out", [1, c.DPC * c.W], F32, isOutput=True)

    # ---- internal DRAM
    g_shard = nc.dram_tensor("g_shard", [c.RCP, 8], F32)
    table = nc.dram_tensor("g_table", [c.TROWS, 8], F32, addr_space="Shared")
    xn_dram = nc.dram_tensor("xn_dram", [c.RCP, 32], F32)
    ys_in = nc.dram_tensor("ys_in", [32, 1], F32)
    ys_out = nc.dram_tensor("ys_out", [32, 1], F32, addr_space="Shared")

    RG = [list(range(c.NC))]

    with tile.TileContext(nc) as tc:
        with (
            tc.tile_pool(name="persist", bufs=1) as P,
            tc.tile_pool(name="stream2", bufs=2) as S2,
            tc.tile_pool(name="stream3", bufs=3) as S3,
            tc.tile_pool(name="psmm", bufs=2, space="PSUM") as PM,
            tc.tile_pool(name="pstp", bufs=2, space="PSUM") as PT,
            tc.tile_pool(name="pssm", bufs=2, space="PSUM") as PS,
        ):
            # ---------- phase A: small persistent loads
            ident = P.tile([128, 128], F32, tag="ident")
            nc.gpsimd.memset(ident[:, :], 0.0)
            nc.gpsimd.affine_select(
                out=ident[:, :], in_=ident[:, :],
                compare_op=ALU.not_equal, fill=1.0, base=0,
                pattern=[[-1, 128]], channel_multiplier=1)
            ipm = P.tile([128, c.NBLK], I32, tag="ipm")
            nc.sync.dma_start(out=ipm[:, :], in_=ipm_d[:, :])
            dgg = P.tile([128, c.SBLK * 32], F32, tag="dgg")
            nc.sync.dma_start(out=dgg[:, :], in_=dgg_d[:, :])
            dgp = P.tile([128, c.NBLK], F32, tag="dgp")
            nc.sync.dma_start(out=dgp[:, :], in_=dgp_d[:, :])
            wt = {}
            for name, arr in w.items():
                t = P.tile(list(arr.shape), F32, tag="w_" + name)
                nc.sync.dma_start(out=t[:, :], in_=wd[name][:, :])
                wt[name] = t
            # dinv in place: x -> 1/x -> sqrt
            nc.vector.reciprocal(dgg[:, :], dgg[:, :])
            nc.scalar.activation(dgg[:, :], dgg[:, :], ACT_T.Sqrt)
            nc.vector.reciprocal(dgp[:, :], dgp[:, :])
            nc.scalar.activation(dgp[:, :], dgp[:, :], ACT_T.Sqrt)

            # zero pad row of table (host-supplied zeros -> no producer waits)
            zdma = nc.sync.dma_start(out=table[c.PADROW:c.PADROW + 1, :],
                                     in_=zrow_d[:, :])


            # ---------- phase B+C: mlp1 -> g_sb
            g_sb = P.tile([128, c.SBLK * 32], F32, tag="g_sb")
            for ci, (c0, cw) in enumerate(_ceil_chunks(c.PCK, c.CW)):
                xpc = S2.tile([64, c.CW], F32, tag="xpc")
                nc.sync.dma_start(out=xpc[:, :cw], in_=xp_d[:, c0:c0 + cw])
                ps1 = PM.tile([128, c.CW], F32, tag="mm")
                nc.tensor.matmul(ps1[:, :cw], lhsT=wt["w1b"][:, :], rhs=xpc[:, :cw],
                                 start=True, stop=True)
                t1 = S2.tile([128, c.CW], F32, tag="t1")
                nc.scalar.activation(t1[:, :cw], ps1[:, :cw], ACT_T.Relu, bias=wt["b1p"][:, :])
                ps2 = PM.tile([128, c.CW], F32, tag="mm")
                nc.tensor.matmul(ps2[:64, :cw], lhsT=wt["w2b"][:, :], rhs=t1[:, :cw],
                                 start=True, stop=True)
                t2 = S2.tile([64, c.CW], F32, tag="t2")
                nc.scalar.activation(t2[:, :cw], ps2[:64, :cw], ACT_T.Relu, bias=wt["b2p"][:, :])
                ps3 = PM.tile([128, c.CW], F32, tag="mm")
                nc.tensor.matmul(ps3[:32, :cw], lhsT=wt["w3b"][:, :], rhs=t2[:, :cw],
                                 start=True, stop=True)
                hch = S2.tile([32, c.CW], F32, tag="hch")
                nc.scalar.activation(hch[:, :cw], ps3[:32, :cw], ACT_T.Identity,
                                     bias=wt["b3p"][:, :])
                for sub in range(cw // 128):
                    cb = (c0 // 128) + sub
                    pt = PT.tile([128, 128], F32, tag="tp")
                    nc.tensor.transpose(pt[:, :32], hch[:, sub * 128:(sub + 1) * 128],
                                        ident[:32, :32])
                    gsv = g_sb[:, :].rearrange("j (cs cb d) -> j cs cb d",
                                               cs=4, cb=c.SBLK)[:, :, cb, :]
                    nc.vector.tensor_tensor(
                        out=gsv,
                        in0=pt[:, :32].rearrange("j (cs d) -> j cs d", cs=4),
                        in1=dgg[:, cb * 32:(cb + 1) * 32]
                            .rearrange("j (cs d) -> j cs d", cs=4),
                        op=ALU.mult)

            # ---------- phase D: shard -> DRAM -> AllGather
            gdma = nc.sync.dma_start(
                out=g_shard.ap().rearrange("(cs cb j) d -> j (cs cb) d",
                                           cs=4, cb=c.SBLK, j=128),
                in_=g_sb[:, :].rearrange("j (cc d) -> j cc d", d=8))
            ag = nc.gpsimd.collective_compute(
                "AllGather", ALU.bypass,
                ins=[g_shard.ap().opt()],
                outs=[table[0:c.PADROW, :].opt()],
                replica_groups=RG)
            add_dep_helper(ag.ins, gdma.ins)

            # ---------- phase E: gather + segment reduce
            aggr = P.tile([128, c.NBLK * 8], F32, tag="aggr")
            for (b0, nb, coff, ncols) in chunks:
                idxc = S3.tile([128, c.CWG], I32, tag="idxc")
                nc.sync.dma_start(out=idxc[:, :ncols], in_=idx_d[:, coff:coff + ncols])
                gt = S2.tile([128, c.CWG * 8], F32, tag="gt")
                gi = nc.gpsimd.indirect_dma_start(
                    out=gt[:, :ncols * 8], out_offset=None,
                    in_=table[:, :],
                    in_offset=bass.IndirectOffsetOnAxis(ap=idxc[:, :ncols], axis=0))
                add_dep_helper(gi.ins, ag.ins)
                add_dep_helper(gi.ins, zdma.ins)
                off = 0
                for b in range(b0, b0 + nb):
                    kbb = int(kb[b])
                    view = gt[:, off * 8:(off + kbb) * 8].rearrange(
                        "p (s d) -> p d s", d=8)
                    col = ((b % c.SBLK) * 4 + (b // c.SBLK)) * 8
                    nc.vector.tensor_reduce(
                        out=aggr[:, col:col + 8], in_=view,
                        axis=AX.X, op=ALU.add)
                    off += kbb
            # dinv scale (broadcast 8 along free)
            nc.vector.tensor_tensor(
                out=aggr[:, :],
                in0=aggr[:, :],
                in1=dgp[:, :].to_broadcast([128, c.NBLK, 8]),
                op=ALU.mult)

            # ---------- phase F+G+H: mlp2 chunks + write xn_dram
            xnv = xn_dram.ap().rearrange("(cs cb j) f -> j cb cs f",
                                         cs=4, cb=c.SBLK, j=128)
            hdmas = []
            for ci, (c0, cw) in enumerate(_ceil_chunks(c.PCK, c.CW)):
                m2c = S2.tile([32, c.CW], F32, tag="m2c")
                for sub in range(cw // 128):
                    cc = (c0 // 128) + sub
                    av = aggr[:, cc * 32:(cc + 1) * 32]
                    pt = PT.tile([128, 128], F32, tag="tp")
                    nc.tensor.transpose(pt[:32, :], av, ident[:, :])
                    nc.vector.tensor_copy(
                        out=m2c[:, sub * 128:(sub + 1) * 128], in_=pt[:32, :])
                ps1 = PM.tile([128, c.CW], F32, tag="mm")
                nc.tensor.matmul(ps1[:, :cw], lhsT=wt["v1b"][:, :], rhs=m2c[:, :cw],
                                 start=True, stop=True)
                u1 = S2.tile([128, c.CW], F32, tag="u1")
                nc.scalar.activation(u1[:, :cw], ps1[:, :cw], ACT_T.Relu, bias=wt["c1p"][:, :])
                ps2 = PM.tile([128, c.CW], F32, tag="mm")
                nc.tensor.matmul(ps2[:64, :cw], lhsT=wt["v2b"][:, :], rhs=u1[:, :cw],
                                 start=True, stop=True)
                u2 = S2.tile([64, c.CW], F32, tag="u2")
                nc.scalar.activation(u2[:, :cw], ps2[:64, :cw], ACT_T.Relu, bias=wt["c2p"][:, :])
                ps3 = PM.tile([128, c.CW], F32, tag="mm")
                nc.tensor.matmul(ps3[:, :cw], lhsT=wt["v3b"][:, :], rhs=u2[:, :cw],
                                 start=True, stop=True)
                xnp_ = S2.tile([128, c.CW], F32, tag="xnp")
                nc.scalar.activation(xnp_[:, :cw], ps3[:, :cw], ACT_T.Identity,
                                     bias=wt["c3p"][:, :])
                for sub in range(cw // 128):
                    cb = (c0 // 128) + sub
                    pt = PT.tile([128, 128], F32, tag="tp")
                    nc.tensor.transpose(pt[:, :], xnp_[:, sub * 128:(sub + 1) * 128],
                                        ident[:, :])
                    stg = S3.tile([128, 128], F32, tag="stg")
                    nc.vector.tensor_copy(out=stg[:, :], in_=pt[:, :])
                    hd = nc.sync.dma_start(
                        out=xnv[:, cb, :, :],
                        in_=stg[:, :].rearrange("j (cs f) -> j cs f", cs=4))
                    hdmas.append(hd)

            # ---------- phase I+J: un-permute gather + build xn_t
            xn_t = P.tile([128, c.OPSW], F32, tag="xn_t")
            for q in range(4):
                xo = S2.tile([128, c.SBLK * 32], F32, tag="xo")
                gq = nc.gpsimd.indirect_dma_start(
                    out=xo[:, :], out_offset=None,
                    in_=xn_dram[:, :],
                    in_offset=bass.IndirectOffsetOnAxis(
                        ap=ipm[:, q * c.SBLK:(q + 1) * c.SBLK], axis=0))
                for hd in hdmas:
                    add_dep_helper(gq.ins, hd.ins)
                for cc in range(c.SBLK):
                    b = q * c.SBLK + cc
                    pt = PT.tile([128, 128], F32, tag="tp")
                    nc.tensor.transpose(pt[:32, :], xo[:, cc * 32:(cc + 1) * 32],
                                        ident[:, :])
                    left = 0
                    while left < 128:
                        n = b * 128 + left
                        s = n // c.OPSW
                        if s >= 4:
                            break
                        ccol = n - s * c.OPSW
                        span = min(128 - left, c.OPSW - ccol)
                        nc.vector.tensor_copy(
                            out=xn_t[32 * s:32 * s + 32, ccol:ccol + span],
                            in_=pt[:32, left:left + span])
                        left += span
            nreal = c.RC - 3 * c.OPSW
            if nreal < c.OPSW:
                nc.vector.memset(xn_t[96:128, nreal:c.OPSW], 0.0)

            # ---------- phase K: y
            r1 = P.tile([64, c.SDAG], F32, tag="r1")
            xy_cw = c.OPSW // 4 if (c.OPSW // 4) % c.OPP == 0 else c.OPSW
            for i, (c0, cw) in enumerate(_ceil_chunks(c.OPSW, xy_cw)):
                nd = cw // c.OPP
                xyc = S2.tile([64, xy_cw], F32, tag="xyc")
                nc.sync.dma_start(out=xyc[:, :cw], in_=xy_d[:, c0:c0 + cw])
                nc.vector.tensor_reduce(
                    out=r1[:, c0 // c.OPP:c0 // c.OPP + nd],
                    in_=xyc[:, :cw].rearrange("p (g o) -> p g o", o=c.OPP),
                    axis=AX.X, op=ALU.add)
            r2 = P.tile([128, c.SDAG], F32, tag="r2")
            nc.vector.tensor_reduce(
                out=r2[:, :],
                in_=xn_t[:, :].rearrange("p (g o) -> p g o", o=c.OPP),
                axis=AX.X, op=ALU.add)
            yin = P.tile([48, c.DPC], F32, tag="yin")
            for s in range(4):
                nd = min(c.SDAG, c.DPC - s * c.SDAG)
                if nd <= 0:
                    continue
                nc.sync.dma_start(out=yin[0:16, s * c.SDAG:s * c.SDAG + nd],
                                  in_=r1[16 * s:16 * s + 16, :nd])
                nc.sync.dma_start(out=yin[16:48, s * c.SDAG:s * c.SDAG + nd],
                                  in_=r2[32 * s:32 * s + 32, :nd])
            psk = PS.tile([128, c.DPC], F32, tag="sm")
            nc.tensor.matmul(psk[:32, :], lhsT=wt["d1"][:, :], rhs=yin[:, :],
                             start=True, stop=True)
            q1 = P.tile([32, c.DPC], F32, tag="q1")
            nc.scalar.activation(q1[:, :], psk[:32, :], ACT_T.Relu, bias=wt["db1"][:, :])
            psk2 = PS.tile([128, c.DPC], F32, tag="sm")
            nc.tensor.matmul(psk2[:16, :], lhsT=wt["d2"][:, :], rhs=q1[:, :],
                             start=True, stop=True)
            q2 = P.tile([16, c.DPC], F32, tag="q2")
            nc.scalar.activation(q2[:, :], psk2[:16, :], ACT_T.Relu, bias=wt["db2"][:, :])
            psk3 = PS.tile([128, c.DPC], F32, tag="sm")
            nc.tensor.matmul(psk3[:32, :], lhsT=wt["d3"][:, :], rhs=q2[:, :],
                             start=True, stop=True)
            y_t = P.tile([32, c.DPC], F32, tag="y_t")
            nc.scalar.activation(y_t[:, :], psk3[:32, :], ACT_T.Identity, bias=wt["db3"][:, :])

            # ---------- phase L: z (AllReduce of ysum)
            ysum = P.tile([32, 1], F32, tag="ysum")
            nc.vector.tensor_reduce(out=ysum[:, :], in_=y_t[:, :], axis=AX.X, op=ALU.add)
            yd1 = nc.sync.dma_start(out=ys_in[:, :], in_=ysum[:, :])
            ar = nc.gpsimd.collective_compute(
                "AllReduce", ALU.add,
                ins=[ys_in.ap().opt()], outs=[ys_out.ap().opt()],
                replica_groups=RG)
            add_dep_helper(ar.ins, yd1.ins)
            ysg = P.tile([32, 1], F32, tag="ysg")
            yd2 = nc.sync.dma_start(out=ysg[:, :], in_=ys_out[:, :])
            add_dep_helper(yd2.ins, ar.ins)
            pse = PS.tile([128, c.DPC], F32, tag="sm")
            nc.tensor.matmul(pse[:32, :1], lhsT=wt["g1"][:, :], rhs=ysg[:, :],
                             start=True, stop=True)
            e1 = P.tile([32, 1], F32, tag="e1")
            nc.scalar.activation(e1[:, :], pse[:32, :1], ACT_T.Relu, bias=wt["gb1"][:, :])
            pse2 = PS.tile([128, c.DPC], F32, tag="sm")
            nc.tensor.matmul(pse2[:16, :1], lhsT=wt["g2"][:, :], rhs=e1[:, :],
                             start=True, stop=True)
            e2 = P.tile([16, 1], F32, tag="e2")
            nc.scalar.activation(e2[:, :], pse2[:16, :1], ACT_T.Relu, bias=wt["gb2"][:, :])
            pse3 = PS.tile([128, c.DPC], F32, tag="sm")
            nc.tensor.matmul(pse3[:32, :1], lhsT=wt["g3"][:, :], rhs=e2[:, :],
                             start=True, stop=True)
            z_t = P.tile([32, 1], F32, tag="z_t")
            nc.scalar.activation(z_t[:, :], pse3[:32, :1], ACT_T.Identity, bias=wt["gb3"][:, :])

            # ---------- phase M: ops
            psy = PS.tile([128, c.DPC], F32, tag="sm")
            nc.tensor.matmul(psy[:32, :], lhsT=wt["ob"][:, :], rhs=y_t[:, :],
                             start=True, stop=True)
            ybp = P.tile([32, c.DPC], F32, tag="ybp")
            nc.vector.tensor_copy(out=ybp[:, :], in_=psy[:32, :])
            psz = PS.tile([128, c.DPC], F32, tag="sm")
            nc.tensor.matmul(psz[:32, :1], lhsT=wt["oc"][:, :], rhs=z_t[:, :],
                             start=True, stop=True)
            opb = P.tile([32, 1], F32, tag="opb")
            nc.vector.tensor_tensor(out=opb[:, :], in0=psz[:32, :1], in1=wt["ob1"][:, :],
                                    op=ALU.add)
            opbr = P.tile([128, 1], F32, tag="opbr")
            for s in range(4):
                nc.vector.tensor_copy(out=opbr[32 * s:32 * s + 32, :], in_=opb[:, :])
            for i, (c0, cw) in enumerate(_ceil_chunks(c.OPSW, 2 * c.OPP)):
                nd = cw // c.OPP
                pso = PM.tile([128, c.CW], F32, tag="mm")
                nc.tensor.matmul(pso[:, :cw], lhsT=wt["oab"][:, :], rhs=xn_t[:, c0:c0 + cw],
                                 start=True, stop=True)
                for s in range(4):
                    d0 = s * c.SDAG + i * 2
                    nds = max(0, min(nd, c.DPC - d0))
                    if nds == 0:
                        continue
                    nc.vector.tensor_tensor(
                        out=pso[32 * s:32 * s + 32, :nds * c.OPP],
                        in0=pso[32 * s:32 * s + 32, :nds * c.OPP],
                        in1=ybp[:, d0:d0 + nds].to_broadcast([32, nds, c.OPP]),
                        op=ALU.add)
                to1 = S2.tile([128, c.CW], F32, tag="to1")
                nc.scalar.activation(to1[:, :cw], pso[:, :cw], ACT_T.Relu, bias=opbr[:, :])
                pso2 = PM.tile([128, c.CW], F32, tag="mm")
                nc.tensor.matmul(pso2[:64, :cw], lhsT=wt["o2b"][:, :], rhs=to1[:, :cw],
                                 start=True, stop=True)
                to2 = S2.tile([64, c.CW], F32, tag="to2")
                nc.scalar.activation(to2[:, :cw], pso2[:64, :cw], ACT_T.Relu, bias=wt["ob2"][:, :])
                pso3 = PM.tile([128, c.CW], F32, tag="mm")
                nc.tensor.matmul(pso3[:4, :cw], lhsT=wt["o3b"][:, :], rhs=to2[:, :cw],
                                 start=True, stop=True)
                opsc = S2.tile([4, 2 * c.OPP], F32, tag="opsc")
                nc.scalar.activation(opsc[:, :cw], pso3[:4, :cw], ACT_T.Identity,
                                     bias=wt["ob3"][:, :])
                nc.sync.dma_start(out=ops_d[:, c0:c0 + cw], in_=opsc[:, :cw])

            # ---------- phase N: prlvl
            psp = PS.tile([128, c.DPC], F32, tag="sm")
            nc.tensor.matmul(psp[:32, :], lhsT=wt["p1y"][:, :], rhs=y_t[:, :],
                             start=True, stop=True)
            pyp = P.tile([32, c.DPC], F32, tag="pyp")
            nc.vector.tensor_copy(out=pyp[:, :], in_=psp[:32, :])
            psz2 = PS.tile([128, c.DPC], F32, tag="sm")
            nc.tensor.matmul(psz2[:32, :1], lhsT=wt["p1z"][:, :], rhs=z_t[:, :],
                             start=True, stop=True)
            pb1f = P.tile([32, 1], F32, tag="pb1f")
            nc.vector.tensor_tensor(out=pb1f[:, :], in0=psz2[:32, :1], in1=wt["pb1"][:, :],
                                    op=ALU.add)
            for i, (c0, cw) in enumerate(_ceil_chunks(c.DPC * c.W, 10 * c.W)):
                nd = cw // c.W
                limc = S3.tile([1, 10 * c.W], F32, tag="limc")
                nc.gpsimd.iota(limc[:1, :cw].rearrange("p (g w) -> p g w", w=c.W),
                               pattern=[[0, nd], [1, c.W]], base=1,
                               channel_multiplier=0,
                               allow_small_or_imprecise_dtypes=True)
                psq = PM.tile([128, c.CW], F32, tag="mm")
                nc.tensor.matmul(psq[:32, :cw], lhsT=wt["p1l"][:, :], rhs=limc[:1, :cw],
                                 start=True, stop=True)
                d0 = i * 10
                nc.vector.tensor_tensor(
                    out=psq[:32, :cw], in0=psq[:32, :cw],
                    in1=pyp[:, d0:d0 + nd].to_broadcast([32, nd, c.W]),
                    op=ALU.add)
                tp1 = S2.tile([32, c.CW], F32, tag="tp1")
                nc.scalar.activation(tp1[:, :cw], psq[:32, :cw], ACT_T.Relu, bias=pb1f[:, :])
                psq2 = PM.tile([128, c.CW], F32, tag="mm")
                nc.tensor.matmul(psq2[:16, :cw], lhsT=wt["p2"][:, :], rhs=tp1[:, :cw],
                                 start=True, stop=True)
                tp2 = S2.tile([16, c.CW], F32, tag="tp2")
                nc.scalar.activation(tp2[:, :cw], psq2[:16, :cw], ACT_T.Relu, bias=wt["pb2"][:, :])
                psq3 = PM.tile([128, c.CW], F32, tag="mm")
                nc.tensor.matmul(psq3[:1, :cw], lhsT=wt["p3"][:, :], rhs=tp2[:, :cw],
                                 start=True, stop=True)
                prlc = S2.tile([1, 10 * c.W], F32, tag="prlc")
                nc.scalar.activation(prlc[:, :cw], psq3[:1, :cw], ACT_T.Identity,
                                     bias=wt["pb3"][:, :])
                nc.sync.dma_start(out=prl_d[:, c0:c0 + cw], in_=prlc[:, :cw])

    nc.finalize()
    return nc


def _run(cfg, prep, trace=False):
    c = cfg
    nc = _build(cfg, prep)
    w = prep["wts"]
    in_maps = []
    for k in range(c.NC):
        pc = prep["per_core"][k]
        m = dict(
            x_pack=pc["x_pack"], x_y=pc["x_y"], idx=pc["idx"], iperm=pc["iperm"],
            deg_gb8=pc["deg_gb8"], deg_pb=pc["deg_pb"],
            zrow=np.zeros((1, 8), np.float32),
        )
        for name, arr in w.items():
            m["w_" + name] = np.ascontiguousarray(arr, np.float32)
        in_maps.append(m)
    res = run_bass_kernel_spmd(nc, in_maps, core_ids=list(range(c.NC)), trace=trace)
    ops = np.concatenate(
        [res.results[k]["ops_out"].reshape(-1)[:c.RC] for k in range(c.NC)])
    prl = np.concatenate(
        [res.results[k]["prl_out"].reshape(c.DPC, c.W) for k in range(c.NC)])
    return ops, prl, res


def kernel(**inputs):
    cfg = Cfg()
    x = np.asarray(inputs["x"], np.float32)
    ei = np.asarray(inputs["edge_index"])
    weights = (
        [np.asarray(a, np.float32) for a in inputs["p_mlp1"]],
        [np.asarray(a, np.float32) for a in inputs["p_mlp2"]],
        [np.asarray(a, np.float32) for a in inputs["p_dag"]],
        [np.asarray(a, np.float32) for a in inputs["p_glob"]],
        [np.asarray(a, np.float32) for a in inputs["p_op"]],
        [np.asarray(a, np.float32) for a in inputs["p_prlvl"]],
    )
    prep = _prep(cfg, x, ei, weights)
    ops, prl, _ = _run(cfg, prep, trace=False)
    return ops, prl
